# revision 1
# baseline (speedup 1.0000x reference)
"""Griffin-Lim phase reconstruction on Trainium2 (Bass/Tile).

Key observations exploited here:
  * The reference returns only wav[:, 15:1015] -- the first 1000 samples of a
    32224-sample overlap-add waveform.  Influence propagates at most +-7
    frames per Griffin-Lim iteration and is strongly attenuated by the Hann
    window tails, so only the first TC=80 (of 1000) STFT frames can affect
    the output (validated numerically: rel err ~5e-6 = the fp32 noise floor;
    the cliff is below 48 frames).
  * The phase never needs to be materialized: carrying (mag*cos, mag*sin) and
    renormalizing with Re/|z|, Im/|z| reproduces angle()+exp() exactly, so no
    atan2/sin/cos in the loop (and |z| errors do not accumulate because the
    next iteration rescales by the fixed magnitude anyway).
  * irfft/rfft of 256 points are dense fp32 matmuls.  Per iteration only 8
    K=128 matmuls run:
      - ISTFT: 4 matmuls produce the windowed frames in (sample, frame)
        layout (two 128-sample halves x {cos,sin} basis accumulated in PSUM);
        the overlap-add collapses the 8 hop-shifted 32-partition groups with
        in-place DVE adds (partition-base-shifted PSUM operands).
      - STFT: the frame gather is 8 hop-shifted copies of the waveform tile
        into two 128-partition operands (DVE/ScalarE partition-shifted
        copies), then 4 matmuls against the windowed DFT basis.
  * ScalarE executes only Sqrt (single LUT, no table switches); everything
    else elementwise runs on VectorE.

Layouts (per core; core c handles batch element c%4, cores 4-7 duplicate):
  SA[128, 7+TC+7]  spec chunk A: rows f=0..127 of mag*cos(theta), zero-padded
  SB[...]          chunk B: row0 = Nyquist mag*cos, rows 1..127 = mag*sin
  P1/P2[128, TC]   frames: P1[n,m]/P2[128+n,m] = windowed sample n of frame m
  Wn[32, TC]       waveform as wav[32*m + i] at (partition i, column m)
  T2R/T2I[128, TS] STFT output: T2R = Re[f=0..127]; T2I row0 = Re[Nyquist],
                   rows 1..127 = Im[f=1..127]   (Im at f=0 and Nyquist == 0)
"""

import numpy as np
from contextlib import ExitStack

import concourse.bass as bass
import concourse.tile as tile
from concourse import bacc, mybir
from concourse import bass_utils

F32 = mybir.dt.float32
AF = mybir.ActivationFunctionType
OP = mybir.AluOpType

TC = 80           # cropped frame count (of 1000)
TS = TC - 7       # stft / phase-update frame count
PAD = 7
N_ITER = 32
N_FFT = 256
NF = 129
HOP = 32
N_CORES = 8
B = 4


def _consts():
    n = np.arange(N_FFT, dtype=np.float64)
    win = 0.5 - 0.5 * np.cos(2.0 * np.pi * n / N_FFT)
    k = np.arange(128, dtype=np.float64)[:, None]
    ang = 2.0 * np.pi * k * n[None, :] / N_FFT
    ck = np.where(k == 0, 1.0, 2.0) / N_FFT
    a_r = (ck * np.cos(ang) * win[None, :]).astype(np.float32)       # (128,256)
    a_i = (-2.0 / N_FFT * np.sin(ang) * win[None, :]).astype(np.float32)
    a_i[0] = (np.cos(np.pi * n) / N_FFT * win).astype(np.float32)    # Nyquist row

    f = np.arange(128, dtype=np.float64)[None, :]
    ang2 = 2.0 * np.pi * f * n[:, None] / N_FFT                      # (256,128)
    bc = (win[:, None] * np.cos(ang2)).astype(np.float32)
    bi = (-win[:, None] * np.sin(ang2)).astype(np.float32)
    bi[:, 0] = (win * np.cos(np.pi * n)).astype(np.float32)

    L = TC * HOP
    wsq = np.zeros((TC + 8) * HOP + N_FFT, dtype=np.float64)
    w2 = win ** 2
    for t in range(TC + 8):
        s = t * HOP
        wsq[s:s + N_FFT] += w2
    wsq = np.maximum(wsq[:L], 1e-8)
    invwsq = (1.0 / wsq).astype(np.float32).reshape(TC, HOP).T.copy()  # (32, TC)
    return a_r, a_i, bc.copy(), bi.copy(), invwsq


def _emit(tc_ctx, aps, rep=1):
    tc = tc_ctx
    nc = tc.nc
    with ExitStack() as ctx:
        consts = ctx.enter_context(tc.tile_pool(name="consts", bufs=1))
        state = ctx.enter_context(tc.tile_pool(name="state", bufs=1))
        work = ctx.enter_context(tc.tile_pool(name="work", bufs=3))
        psum = ctx.enter_context(tc.tile_pool(name="psum", bufs=2, space="PSUM"))

        a_r = consts.tile([128, 256], F32)
        a_i = consts.tile([128, 256], F32)
        bca = consts.tile([128, 128], F32)
        bcb = consts.tile([128, 128], F32)
        bia = consts.tile([128, 128], F32)
        bib = consts.tile([128, 128], F32)
        invw = consts.tile([32, TC], F32)
        maga = consts.tile([128, TS], F32)
        magn = consts.tile([1, TS], F32)
        sa = state.tile([128, TC + 2 * PAD], F32)
        sb = state.tile([128, TC + 2 * PAD], F32)
        epsb = consts.tile([128, 1], F32)
        nc.vector.memset(epsb, 1e-6)

        for t, name in [(a_r, "a_r"), (a_i, "a_i"), (bca, "bca"), (bcb, "bcb"),
                        (bia, "bia"), (bib, "bib"),
                        (invw, "invw"), (maga, "maga"), (magn, "magn")]:
            nc.sync.dma_start(out=t, in_=aps[name])

        if rep > 1:
            from concourse.engine_type import EngineType
            loop = tc.For_i(0, rep, 1, hint_engines=(
                EngineType.PE, EngineType.DVE, EngineType.Activation,
                EngineType.SP))
        else:
            loop = None
        if loop is not None:
            loop.__enter__()
        nc.sync.dma_start(out=sa, in_=aps["sa0"])
        nc.sync.dma_start(out=sb, in_=aps["sb0"])

        for it in range(N_ITER):
            last = it == N_ITER - 1
            # ---- ISTFT: frames in (n, m) layout via 4 K=128 matmuls ----
            p1 = psum.tile([128, TC], F32, tag="p1")   # samples n=0..127
            p2 = psum.tile([128, TC], F32, tag="p2")   # samples n=128..255
            nc.tensor.matmul(p1, a_r[:, 0:128], sa[:, PAD:PAD + TC],
                             start=True, stop=False)
            nc.tensor.matmul(p2, a_r[:, 128:256], sa[:, PAD:PAD + TC],
                             start=True, stop=False)
            nc.tensor.matmul(p1, a_i[:, 0:128], sb[:, PAD:PAD + TC],
                             start=False, stop=True)
            nc.tensor.matmul(p2, a_i[:, 128:256], sb[:, PAD:PAD + TC],
                             start=False, stop=True)
            # ---- overlap-add: shifted partition-group accumulation ----
            # (walrus requires equal SB base partitions for 2-SB-input ops,
            #  so accumulate sequentially with the PSUM operand shifted)
            wn = work.tile([32, TC], F32, tag="wn")
            nc.scalar.copy(wn, p1[0:32, :])
            for j in range(1, 4):
                nc.vector.tensor_add(wn[:, j:TC], wn[:, j:TC],
                                     p1[32 * j:32 * j + 32, 0:TC - j])
            for j in range(4, 8):
                nc.vector.tensor_add(wn[:, j:TC], wn[:, j:TC],
                                     p2[32 * (j - 4):32 * (j - 4) + 32, 0:TC - j])
            nc.vector.tensor_mul(wn, wn, invw)

            if last:
                nc.sync.dma_start(out=aps["out"], in_=wn[:, 0:32])
                break

            # ---- STFT: build hop-shifted frame gather via partition copies ----
            ga = work.tile([128, TS], F32, tag="ga")
            gb = work.tile([128, TS], F32, tag="gb")
            nc.vector.tensor_copy(ga[0:32, :], wn[:, 0:TS])
            nc.scalar.copy(ga[32:64, :], wn[:, 1:1 + TS])
            nc.vector.tensor_copy(ga[64:96, :], wn[:, 2:2 + TS])
            nc.scalar.copy(ga[96:128, :], wn[:, 3:3 + TS])
            t2r = psum.tile([128, TS], F32, tag="t2r")
            t2i = psum.tile([128, TS], F32, tag="t2i")
            nc.tensor.matmul(t2r, bca, ga, start=True, stop=False)
            nc.tensor.matmul(t2i, bia, ga, start=True, stop=False)
            nc.vector.tensor_copy(gb[0:32, :], wn[:, 4:4 + TS])
            nc.scalar.copy(gb[32:64, :], wn[:, 5:5 + TS])
            nc.vector.tensor_copy(gb[64:96, :], wn[:, 6:6 + TS])
            nc.scalar.copy(gb[96:128, :], wn[:, 7:7 + TS])
            nc.tensor.matmul(t2r, bcb, gb, start=False, stop=True)
            nc.tensor.matmul(t2i, bib, gb, start=False, stop=True)

            # ---- phase update: z/|z| carried as (cos, sin) ----
            # ACT runs ONLY Sqrt (one LUT, never switches); rest on DVE.
            rA = work.tile([128, TS], F32, tag="rA")
            iA = work.tile([128, TS], F32, tag="iA")
            nc.vector.tensor_scalar_add(rA, t2r, 1e-6)
            nc.scalar.copy(iA, t2i)
            nc.vector.memset(iA[0:1, :], 0.0)   # Im at DC is exactly 0
            sq = work.tile([128, TS], F32, tag="sq")
            sqi = work.tile([128, TS], F32, tag="sqi")
            nc.vector.tensor_mul(sq, rA, rA)
            nc.vector.tensor_mul(sqi, iA, iA)
            nc.vector.tensor_add(sq, sq, sqi)
            hyp = work.tile([128, TS], F32, tag="hyp")
            nc.scalar.activation(hyp, sq, AF.Sqrt)
            inv = work.tile([128, TS], F32, tag="inv")
            nc.vector.reciprocal(inv, hyp)
            pm = work.tile([128, TS], F32, tag="pm")
            nc.vector.tensor_mul(pm, maga, inv)
            nc.vector.tensor_mul(sa[:, PAD:PAD + TS], rA, pm)
            nc.vector.tensor_mul(sb[:, PAD:PAD + TS], iA, pm)
            # Nyquist row (sb row 0): Im==0 there, so value is mag*sign(Re+eps)
            ge = work.tile([1, TS], F32, tag="ge")
            nc.vector.tensor_scalar(ge, t2i[0:1, :], -1e-6, 2.0,
                                    OP.is_ge, OP.mult)
            nc.vector.scalar_tensor_tensor(sb[0:1, PAD:PAD + TS], ge, 1.0,
                                           magn, OP.subtract, OP.mult)
        if loop is not None:
            loop.__exit__(None, None, None)


_CACHED = None


def _build(rep=1):
    global _CACHED
    if rep == 1 and _CACHED is not None:
        return _CACHED
    nc = bacc.Bacc("TRN2", target_bir_lowering=False, debug=False,
                   num_devices=N_CORES)
    shapes = {
        "a_r": (128, 256), "a_i": (128, 256), "bca": (128, 128),
        "bcb": (128, 128), "bia": (128, 128), "bib": (128, 128),
        "invw": (32, TC), "maga": (128, TS),
        "magn": (1, TS), "sa0": (128, TC + 2 * PAD), "sb0": (128, TC + 2 * PAD),
    }
    aps = {name: nc.dram_tensor(name, shape, F32, kind="ExternalInput").ap()
           for name, shape in shapes.items()}
    aps["out"] = nc.dram_tensor("out", (32, 32), F32, kind="ExternalOutput").ap()
    with tile.TileContext(nc) as t:
        _emit(t, aps, rep=rep)
    nc.compile()
    if rep == 1:
        _CACHED = nc
    return nc


def _host_inputs(mag_b, ph_b):
    """Per-batch host prep: crop, initial cos/sin spec chunks, padding."""
    a_r, a_i, bc, bi, invwsq = _consts()
    mag = np.ascontiguousarray(mag_b[:, :TC]).astype(np.float32)
    ph = np.ascontiguousarray(ph_b[:, :TC]).astype(np.float32)
    sa0 = np.zeros((128, TC + 2 * PAD), np.float32)
    sb0 = np.zeros((128, TC + 2 * PAD), np.float32)
    sa0[:, PAD:PAD + TC] = mag[0:128] * np.cos(ph[0:128])
    sb0[0, PAD:PAD + TC] = mag[128] * np.cos(ph[128])
    sb0[1:, PAD:PAD + TC] = mag[1:128] * np.sin(ph[1:128])
    return {
        "a_r": a_r, "a_i": a_i,
        "bca": np.ascontiguousarray(bc[0:128]), "bcb": np.ascontiguousarray(bc[128:256]),
        "bia": np.ascontiguousarray(bi[0:128]), "bib": np.ascontiguousarray(bi[128:256]),
        "invw": invwsq,
        "maga": np.ascontiguousarray(mag[0:128, :TS]),
        "magn": np.ascontiguousarray(mag[128:129, :TS]),
        "sa0": sa0, "sb0": sb0,
    }


def kernel(mag_spec, phase):
    mag_spec = np.asarray(mag_spec, dtype=np.float32)
    phase = np.asarray(phase, dtype=np.float32)
    nc = _build()
    in_maps = [_host_inputs(mag_spec[c % B], phase[c % B]) for c in range(N_CORES)]
    res = bass_utils.run_bass_kernel_spmd(nc, in_maps, core_ids=list(range(N_CORES)))
    out = np.zeros((B, 1000), np.float32)
    for b in range(B):
        blk = res.results[b]["out"]              # (32, 32): [i, m] = wav[32m+i]
        out[b] = blk.T.reshape(-1)[15:1015]
    return out



# revision 10
# speedup vs baseline: 4.5293x; 4.5293x over previous
"""Griffin-Lim phase reconstruction on Trainium2 (Bass/Tile) — v2.

Same math as the baseline (see git history / kernel_baseline.py.bak) with a
restructured per-iteration schedule aimed at the instruction-overhead +
cross-engine-sync bound (SEM_DELAY=100ns, ~40 small serial ops dominated the
old 11us/iter):

  * TC=56 frames (down from 80): numerically validated on CPU, rel err 2e-4
    vs the 2e-2 gate (influence of frames >TC on the cropped output decays
    below fp32 noise; the cliff is under 48 frames).
  * ISTFT + overlap-add fused into the PE: 16 narrow matmuls (M=32, K=128)
    accumulate the hop-shifted frame groups directly into one PSUM tile in
    waveform layout wn[i, m] = wav[32m+i].  The old serial 8-step DVE
    overlap-add chain disappears.
  * The periodic part of the 1/sum(win^2) normalization is folded into the
    ISTFT basis columns (it only depends on sample index mod hop); a single
    tiny (32,7) multiply fixes the first 7 boundary columns.
  * matmuls run as float32r (bitcast): 2 cycles/row instead of fp32's 4.
  * STFT: 8 hop-shifted gather copies split DVE/ACT (ACT reads PSUM faster
    than SBUF), then 4 K=128 matmuls + 2 M=1 matmuls for the Nyquist row.
    The Nyquist column of the main t2i stationary is zeroed so the phase
    update needs no row-0 masking (t2i[0] == 0 exactly).
  * Phase update: sq = t2r^2 + t2i^2 (DVE), 1/|z| in ONE ACT op
    (Abs_reciprocal_sqrt — the only LUT the kernel uses), then fused
    scalar_tensor_tensor ops write the new spec chunks.

Layouts (per core; core c handles batch element c%4, cores 4-7 duplicate):
  SA[128, 7+TC]  spec chunk A: rows f=0..127 of mag*cos(theta), 7 leading
                 zero frames so all shifted matmul operands stay in-bounds
  SB[...]        chunk B: row0 = Nyquist mag*cos, rows 1..127 = mag*sin
  wn (PSUM)[32, TC]  waveform wav[32m+i] at (partition i, column m)
  t2r/t2i[128, TS]   STFT: t2r = Re[f=0..127]; t2i rows 1..127 = Im[f=1..127],
                     row 0 = 0 (Nyquist handled via its own 1-row matmul t2n)
"""

import numpy as np
from contextlib import ExitStack

import concourse.bass as bass
import concourse.tile as tile
from concourse import bacc, mybir
from concourse import bass_utils

F32 = mybir.dt.float32
F32R = mybir.dt.float32r
AF = mybir.ActivationFunctionType
OP = mybir.AluOpType

TC = 56           # cropped frame count (of 1000)
TS = TC - 7       # stft / phase-update frame count (odd -- fine for DVE ops)
NW = TC + 2       # ISTFT matmul width: even (fp32r ISA) and >= TC+1 so the
                  # j=7 gather stays in bounds; cols TC..NW-1 come from the
                  # two always-zero tail frames
TP = TS + 1       # STFT matmul width (even for fp32r); col TS is junk that
                  # the phase update never reads
PAD = 7
N_ITER = 32
N_FFT = 256
NF = 129
HOP = 32
N_CORES = 8
B = 4


def _consts():
    n = np.arange(N_FFT, dtype=np.float64)
    win = 0.5 - 0.5 * np.cos(2.0 * np.pi * n / N_FFT)
    k = np.arange(128, dtype=np.float64)[:, None]
    ang = 2.0 * np.pi * k * n[None, :] / N_FFT
    ck = np.where(k == 0, 1.0, 2.0) / N_FFT
    a_r = ck * np.cos(ang) * win[None, :]                     # (128,256)
    a_i = -2.0 / N_FFT * np.sin(ang) * win[None, :]
    a_i[0] = np.cos(np.pi * n) / N_FFT * win                  # Nyquist row

    f = np.arange(128, dtype=np.float64)[None, :]
    ang2 = 2.0 * np.pi * f * n[:, None] / N_FFT               # (256,128)
    bc = win[:, None] * np.cos(ang2)
    bi = -win[:, None] * np.sin(ang2)
    bn = (win * np.cos(np.pi * n))[:, None]                   # (256,1) Nyquist
    bi[:, 0] = 0.0                                            # t2i row0 == 0

    L = TC * HOP
    wsq = np.zeros((TC + 8) * HOP + N_FFT)
    w2 = win ** 2
    for t in range(TC + 8):
        wsq[t * HOP:t * HOP + N_FFT] += w2
    wsq = np.maximum(wsq[:L], 1e-8)
    invw = (1.0 / wsq).reshape(TC, HOP).T                     # (32, TC)
    invw_p = invw[:, 10].copy()                               # periodic col
    fold = invw_p[np.arange(N_FFT) % HOP]
    a_rf = (a_r * fold[None, :]).astype(np.float32)
    a_if = (a_i * fold[None, :]).astype(np.float32)
    corrt = (invw[:, :PAD] / invw_p[:, None]).astype(np.float32)  # (32,7)
    return (a_rf, a_if, bc.astype(np.float32), bi.astype(np.float32),
            bn.astype(np.float32), corrt)


def _emit(tc_ctx, aps, rep=1):
    tc = tc_ctx
    nc = tc.nc
    with ExitStack() as ctx:
        consts = ctx.enter_context(tc.tile_pool(name="consts", bufs=1))
        state = ctx.enter_context(tc.tile_pool(name="state", bufs=1))
        work = ctx.enter_context(tc.tile_pool(name="work", bufs=2))
        psum = ctx.enter_context(tc.tile_pool(name="psum", bufs=2, space="PSUM"))

        a_r = consts.tile([128, 256], F32R)
        a_i = consts.tile([128, 256], F32R)
        bca = consts.tile([128, 128], F32R)
        bcb = consts.tile([128, 128], F32R)
        bia = consts.tile([128, 128], F32R)
        bib = consts.tile([128, 128], F32R)
        bna = consts.tile([128, 1], F32R)
        bnb = consts.tile([128, 1], F32R)
        corrt = consts.tile([32, PAD], F32)
        maga = consts.tile([128, TS], F32)
        magn = consts.tile([1, TS], F32)
        sa = state.tile([128, NW + PAD], F32R)
        sb = state.tile([128, NW + PAD], F32R)
        epsb = consts.tile([128, 1], F32)
        nc.vector.memset(epsb, 1e-12)

        for t, name in [(a_r, "a_r"), (a_i, "a_i"), (bca, "bca"), (bcb, "bcb"),
                        (bia, "bia"), (bib, "bib"), (bna, "bna"), (bnb, "bnb"),
                        (corrt, "corrt"), (maga, "maga"), (magn, "magn")]:
            nc.sync.dma_start(out=t, in_=aps[name])

        if rep > 1:
            from concourse.engine_type import EngineType
            loop = tc.For_i(0, rep, 1, hint_engines=(
                EngineType.PE, EngineType.DVE, EngineType.Activation,
                EngineType.Pool, EngineType.SP))
        else:
            loop = None
        if loop is not None:
            loop.__enter__()
        nc.sync.dma_start(out=sa, in_=aps["sa0"])
        nc.sync.dma_start(out=sb, in_=aps["sb0"])

        for it in range(N_ITER):
            last = it == N_ITER - 1
            # ---- ISTFT + overlap-add fused in PE: 16 matmuls, one PSUM ----
            # wn[i, m] = sum_j sum_f basis[f, 32j+i] * s[f, m-j]
            wn = psum.tile([32, NW], F32, tag="wn")
            for j in range(8):
                nc.tensor.matmul(wn, a_r[:, 32 * j:32 * j + 32],
                                 sa[:, PAD - j:PAD - j + NW],
                                 start=(j == 0), stop=False)
            for j in range(8):
                nc.tensor.matmul(wn, a_i[:, 32 * j:32 * j + 32],
                                 sb[:, PAD - j:PAD - j + NW],
                                 start=False, stop=(j == 7))
            # boundary normalization fix (cols 0..6); rest is folded in basis
            nc.vector.tensor_mul(wn[:, 0:PAD], wn[:, 0:PAD], corrt)

            if last:
                outt = work.tile([32, 32], F32, tag="outt")
                nc.vector.tensor_copy(outt, wn[:, 0:32])
                nc.sync.dma_start(out=aps["out"], in_=outt)
                break

            # ---- STFT gather: 8 hop-shifted copies of wn (DVE + ACT) ----
            ga = work.tile([128, TP], F32R, tag="ga")
            gb = work.tile([128, TP], F32R, tag="gb")
            nc.vector.tensor_copy(ga[0:32, :], wn[:, 0:TP])
            nc.scalar.copy(ga[32:64, :], wn[:, 1:1 + TP])
            nc.vector.tensor_copy(ga[64:96, :], wn[:, 2:2 + TP])
            nc.scalar.copy(ga[96:128, :], wn[:, 3:3 + TP])
            t2r = psum.tile([128, TP], F32, tag="t2r")
            t2i = psum.tile([128, TP], F32, tag="t2i")
            t2n = psum.tile([1, TP], F32, tag="t2n")
            nc.tensor.matmul(t2r, bca, ga, start=True, stop=False)
            nc.tensor.matmul(t2i, bia, ga, start=True, stop=False)
            nc.tensor.matmul(t2n, bna, ga, start=True, stop=False)
            nc.vector.tensor_copy(gb[0:32, :], wn[:, 4:4 + TP])
            nc.scalar.copy(gb[32:64, :], wn[:, 5:5 + TP])
            nc.vector.tensor_copy(gb[64:96, :], wn[:, 6:6 + TP])
            nc.scalar.copy(gb[96:128, :], wn[:, 7:7 + TP])
            nc.tensor.matmul(t2r, bcb, gb, start=False, stop=True)
            nc.tensor.matmul(t2i, bib, gb, start=False, stop=True)
            nc.tensor.matmul(t2n, bnb, gb, start=False, stop=True)

            # ---- phase update: z/|z| carried as (cos, sin) ----
            # (walrus: max one PSUM input per vector op, so t2r/t2i come to
            #  SBUF first -- rA doubles as the +eps'd Re used for sa')
            rA = work.tile([128, TS], F32, tag="rA")
            iA = work.tile([128, TS], F32, tag="iA")
            m1 = work.tile([128, TS], F32, tag="m1")
            m2 = work.tile([128, TS], F32, tag="m2")
            sq = work.tile([128, TS], F32, tag="sq")
            nc.vector.tensor_scalar_add(rA, t2r[:, 0:TS], 1e-6)
            nc.scalar.copy(iA, t2i[:, 0:TS])                  # row0 == 0 by basis
            nc.vector.tensor_mul(m1, rA, rA)
            nc.gpsimd.tensor_mul(m2, iA, iA)
            # Nyquist sign path (off critical path): mag * sign(Re + eps)
            ge = work.tile([1, TS], F32, tag="ge")
            nc.vector.tensor_scalar(ge, t2n[0:1, 0:TS], -1e-6, 2.0, OP.is_ge, OP.mult)
            nc.vector.tensor_add(sq, m1, m2)
            inv = work.tile([128, TS], F32, tag="inv")
            nc.scalar.activation(inv, sq, AF.Abs_reciprocal_sqrt, bias=epsb)
            pm = work.tile([128, TS], F32, tag="pm")
            nc.vector.tensor_mul(pm, maga, inv)
            nc.vector.tensor_mul(sa[:, PAD:PAD + TS], rA, pm)
            nc.gpsimd.tensor_mul(sb[:, PAD:PAD + TS], iA, pm)  # row0 -> 0
            nc.vector.scalar_tensor_tensor(sb[0:1, PAD:PAD + TS], ge, 1.0,
                                           magn, OP.subtract, OP.mult)
        if loop is not None:
            loop.__exit__(None, None, None)


_CACHED = None


def _build(rep=1):
    global _CACHED
    if rep == 1 and _CACHED is not None:
        return _CACHED
    nc = bacc.Bacc("TRN2", target_bir_lowering=False, debug=False,
                   num_devices=N_CORES)
    shapes = {
        "a_r": (128, 256), "a_i": (128, 256), "bca": (128, 128),
        "bcb": (128, 128), "bia": (128, 128), "bib": (128, 128),
        "bna": (128, 1), "bnb": (128, 1), "corrt": (32, PAD),
        "maga": (128, TS), "magn": (1, TS),
        "sa0": (128, NW + PAD), "sb0": (128, NW + PAD),
    }
    MMIN = {"a_r", "a_i", "bca", "bcb", "bia", "bib", "bna", "bnb",
            "sa0", "sb0"}
    aps = {name: nc.dram_tensor(name, shape, F32R if name in MMIN else F32,
                                kind="ExternalInput").ap()
           for name, shape in shapes.items()}
    aps["out"] = nc.dram_tensor("out", (32, 32), F32, kind="ExternalOutput").ap()
    with tile.TileContext(nc) as t:
        _emit(t, aps, rep=rep)
    nc.compile()
    if rep == 1:
        _CACHED = nc
    return nc


def _host_inputs(mag_b, ph_b):
    """Per-batch host prep: crop, initial cos/sin spec chunks, padding."""
    a_r, a_i, bc, bi, bn, corrt = _consts()
    mag = np.ascontiguousarray(mag_b[:, :TC]).astype(np.float32)
    ph = np.ascontiguousarray(ph_b[:, :TC]).astype(np.float32)
    sa0 = np.zeros((128, NW + PAD), np.float32)
    sb0 = np.zeros((128, NW + PAD), np.float32)
    sa0[:, PAD:PAD + TC] = mag[0:128] * np.cos(ph[0:128])
    sb0[0, PAD:PAD + TC] = mag[128] * np.cos(ph[128])
    sb0[1:, PAD:PAD + TC] = mag[1:128] * np.sin(ph[1:128])
    return {
        "a_r": a_r, "a_i": a_i,
        "bca": np.ascontiguousarray(bc[0:128]),
        "bcb": np.ascontiguousarray(bc[128:256]),
        "bia": np.ascontiguousarray(bi[0:128]),
        "bib": np.ascontiguousarray(bi[128:256]),
        "bna": np.ascontiguousarray(bn[0:128]),
        "bnb": np.ascontiguousarray(bn[128:256]),
        "corrt": corrt,
        "maga": np.ascontiguousarray(mag[0:128, :TS]),
        "magn": np.ascontiguousarray(mag[128:129, :TS]),
        "sa0": sa0, "sb0": sb0,
    }


def kernel(mag_spec, phase):
    mag_spec = np.asarray(mag_spec, dtype=np.float32)
    phase = np.asarray(phase, dtype=np.float32)
    nc = _build()
    in_maps = [_host_inputs(mag_spec[c % B], phase[c % B]) for c in range(N_CORES)]
    res = bass_utils.run_bass_kernel_spmd(nc, in_maps, core_ids=list(range(N_CORES)))
    out = np.zeros((B, 1000), np.float32)
    for b in range(B):
        blk = res.results[b]["out"]              # (32, 32): [i, m] = wav[32m+i]
        out[b] = blk.T.reshape(-1)[15:1015]
    return out


# revision 11
# speedup vs baseline: 15.2634x; 3.3699x over previous
"""Griffin-Lim phase reconstruction on Trainium2 (Bass/Tile) — v2.

Same math as the baseline (see git history / kernel_baseline.py.bak) with a
restructured per-iteration schedule aimed at the instruction-overhead +
cross-engine-sync bound (SEM_DELAY=100ns, ~40 small serial ops dominated the
old 11us/iter):

  * TC=56 frames (down from 80): numerically validated on CPU, rel err 2e-4
    vs the 2e-2 gate (influence of frames >TC on the cropped output decays
    below fp32 noise; the cliff is under 48 frames).
  * ISTFT + overlap-add fused into the PE: 16 narrow matmuls (M=32, K=128)
    accumulate the hop-shifted frame groups directly into one PSUM tile in
    waveform layout wn[i, m] = wav[32m+i].  The old serial 8-step DVE
    overlap-add chain disappears.
  * The periodic part of the 1/sum(win^2) normalization is folded into the
    ISTFT basis columns (it only depends on sample index mod hop); a single
    tiny (32,7) multiply fixes the first 7 boundary columns.
  * matmuls run as float32r (bitcast): 2 cycles/row instead of fp32's 4.
  * STFT: 8 hop-shifted gather copies split DVE/ACT (ACT reads PSUM faster
    than SBUF), then 4 K=128 matmuls + 2 M=1 matmuls for the Nyquist row.
    The Nyquist column of the main t2i stationary is zeroed so the phase
    update needs no row-0 masking (t2i[0] == 0 exactly).
  * Phase update: sq = t2r^2 + t2i^2 (DVE), 1/|z| in ONE ACT op
    (Abs_reciprocal_sqrt — the only LUT the kernel uses), then fused
    scalar_tensor_tensor ops write the new spec chunks.

Layouts (per core; core c handles batch element c%4, cores 4-7 duplicate):
  SA[128, 7+TC]  spec chunk A: rows f=0..127 of mag*cos(theta), 7 leading
                 zero frames so all shifted matmul operands stay in-bounds
  SB[...]        chunk B: row0 = Nyquist mag*cos, rows 1..127 = mag*sin
  wn (PSUM)[32, TC]  waveform wav[32m+i] at (partition i, column m)
  t2r/t2i[128, TS]   STFT: t2r = Re[f=0..127]; t2i rows 1..127 = Im[f=1..127],
                     row 0 = 0 (Nyquist handled via its own 1-row matmul t2n)
"""

import numpy as np
from contextlib import ExitStack

import concourse.bass as bass
import concourse.tile as tile
from concourse import bacc, mybir
from concourse import bass_utils

F32 = mybir.dt.float32
F32R = mybir.dt.float32r
AF = mybir.ActivationFunctionType
OP = mybir.AluOpType

TC = 48           # cropped frame count (of 1000)
TS = TC - 7       # stft / phase-update frame count (odd -- fine for DVE ops)
NW = TC + 2       # ISTFT matmul width: even (fp32r ISA) and >= TC+1 so the
                  # j=7 gather stays in bounds; cols TC..NW-1 come from the
                  # two always-zero tail frames
TP = TS + 1       # STFT matmul width (even for fp32r); col TS is junk that
                  # the phase update never reads
PAD = 7
N_ITER = 32
N_FFT = 256
NF = 129
HOP = 32
N_CORES = 8
B = 4


def _consts():
    n = np.arange(N_FFT, dtype=np.float64)
    win = 0.5 - 0.5 * np.cos(2.0 * np.pi * n / N_FFT)
    k = np.arange(128, dtype=np.float64)[:, None]
    ang = 2.0 * np.pi * k * n[None, :] / N_FFT
    ck = np.where(k == 0, 1.0, 2.0) / N_FFT
    a_r = ck * np.cos(ang) * win[None, :]                     # (128,256)
    a_i = -2.0 / N_FFT * np.sin(ang) * win[None, :]
    a_i[0] = np.cos(np.pi * n) / N_FFT * win                  # Nyquist row

    f = np.arange(128, dtype=np.float64)[None, :]
    ang2 = 2.0 * np.pi * f * n[:, None] / N_FFT               # (256,128)
    bc = win[:, None] * np.cos(ang2)
    bi = -win[:, None] * np.sin(ang2)
    bn = (win * np.cos(np.pi * n))[:, None]                   # (256,1) Nyquist
    bi[:, 0] = 0.0                                            # t2i row0 == 0

    L = TC * HOP
    wsq = np.zeros((TC + 8) * HOP + N_FFT)
    w2 = win ** 2
    for t in range(TC + 8):
        wsq[t * HOP:t * HOP + N_FFT] += w2
    wsq = np.maximum(wsq[:L], 1e-8)
    invw = (1.0 / wsq).reshape(TC, HOP).T                     # (32, TC)
    invw_p = invw[:, 10].copy()                               # periodic col
    fold = invw_p[np.arange(N_FFT) % HOP]
    a_rf = (a_r * fold[None, :]).astype(np.float32)
    a_if = (a_i * fold[None, :]).astype(np.float32)
    corrt = (invw[:, :PAD] / invw_p[:, None]).astype(np.float32)  # (32,7)
    return (a_rf, a_if, bc.astype(np.float32), bi.astype(np.float32),
            bn.astype(np.float32), corrt)


def _emit(tc_ctx, aps, rep=1):
    tc = tc_ctx
    nc = tc.nc
    with ExitStack() as ctx:
        consts = ctx.enter_context(tc.tile_pool(name="consts", bufs=1))
        state = ctx.enter_context(tc.tile_pool(name="state", bufs=1))
        work = ctx.enter_context(tc.tile_pool(name="work", bufs=2))
        psum = ctx.enter_context(tc.tile_pool(name="psum", bufs=2, space="PSUM"))

        a_r = consts.tile([128, 256], F32R)
        a_i = consts.tile([128, 256], F32R)
        bca = consts.tile([128, 128], F32R)
        bcb = consts.tile([128, 128], F32R)
        bia = consts.tile([128, 128], F32R)
        bib = consts.tile([128, 128], F32R)
        bna = consts.tile([128, 1], F32R)
        bnb = consts.tile([128, 1], F32R)
        corrt = consts.tile([32, PAD], F32)
        maga = consts.tile([128, TS], F32)
        magn = consts.tile([1, TS], F32)
        sa = state.tile([128, NW + PAD], F32R)
        sb = state.tile([128, NW + PAD], F32R)
        epsb = consts.tile([128, 1], F32)
        nc.vector.memset(epsb, 1e-12)

        for t, name in [(a_r, "a_r"), (a_i, "a_i"), (bca, "bca"), (bcb, "bcb"),
                        (bia, "bia"), (bib, "bib"), (bna, "bna"), (bnb, "bnb"),
                        (corrt, "corrt"), (maga, "maga"), (magn, "magn")]:
            nc.sync.dma_start(out=t, in_=aps[name])

        if rep > 1:
            from concourse.engine_type import EngineType
            loop = tc.For_i(0, rep, 1, hint_engines=(
                EngineType.PE, EngineType.DVE, EngineType.Activation,
                EngineType.Pool, EngineType.SP))
        else:
            loop = None
        if loop is not None:
            loop.__enter__()
        nc.sync.dma_start(out=sa, in_=aps["sa0"])
        nc.sync.dma_start(out=sb, in_=aps["sb0"])

        for it in range(N_ITER):
            last = it == N_ITER - 1
            # ---- ISTFT + overlap-add fused in PE: 16 matmuls, one PSUM ----
            # wn[i, m] = sum_j sum_f basis[f, 32j+i] * s[f, m-j]
            W = 32 if last else NW
            wn = psum.tile([32, NW], F32, tag="wn")
            for j in range(8):
                nc.tensor.matmul(wn[:, 0:W], a_r[:, 32 * j:32 * j + 32],
                                 sa[:, PAD - j:PAD - j + W],
                                 start=(j == 0), stop=False)
            for j in range(8):
                nc.tensor.matmul(wn[:, 0:W], a_i[:, 32 * j:32 * j + 32],
                                 sb[:, PAD - j:PAD - j + W],
                                 start=False, stop=(j == 7))
            # boundary normalization fix (cols 0..6); rest is folded in basis
            nc.vector.tensor_mul(wn[:, 0:PAD], wn[:, 0:PAD], corrt)

            if last:
                outt = work.tile([32, 32], F32, tag="outt")
                nc.vector.tensor_copy(outt, wn[:, 0:32])
                nc.sync.dma_start(out=aps["out"], in_=outt)
                break

            # ---- STFT gather: 8 hop-shifted copies of wn (DVE + ACT) ----
            ga = work.tile([128, TP], F32R, tag="ga")
            gb = work.tile([128, TP], F32R, tag="gb")
            nc.vector.tensor_copy(ga[0:32, :], wn[:, 0:TP])
            nc.scalar.copy(ga[32:64, :], wn[:, 1:1 + TP])
            nc.vector.tensor_copy(ga[64:96, :], wn[:, 2:2 + TP])
            nc.scalar.copy(ga[96:128, :], wn[:, 3:3 + TP])
            t2r = psum.tile([128, TP], F32, tag="t2r")
            t2i = psum.tile([128, TP], F32, tag="t2i")
            t2n = psum.tile([1, TP], F32, tag="t2n")
            nc.tensor.matmul(t2r, bca, ga, start=True, stop=False)
            nc.tensor.matmul(t2i, bia, ga, start=True, stop=False)
            nc.tensor.matmul(t2n, bna, ga, start=True, stop=False)
            nc.vector.tensor_copy(gb[0:32, :], wn[:, 4:4 + TP])
            nc.scalar.copy(gb[32:64, :], wn[:, 5:5 + TP])
            nc.vector.tensor_copy(gb[64:96, :], wn[:, 6:6 + TP])
            nc.scalar.copy(gb[96:128, :], wn[:, 7:7 + TP])
            nc.tensor.matmul(t2r, bcb, gb, start=False, stop=True)
            nc.tensor.matmul(t2i, bib, gb, start=False, stop=True)
            nc.tensor.matmul(t2n, bnb, gb, start=False, stop=True)

            # ---- phase update: z/|z| carried as (cos, sin) ----
            # (walrus: max one PSUM input per vector op, so t2r/t2i come to
            #  SBUF first -- rA doubles as the +eps'd Re used for sa')
            rA = work.tile([128, TS], F32, tag="rA")
            iA = work.tile([128, TS], F32, tag="iA")
            m1 = work.tile([128, TS], F32, tag="m1")
            m2 = work.tile([128, TS], F32, tag="m2")
            sq = work.tile([128, TS], F32, tag="sq")
            nc.vector.tensor_scalar_add(rA, t2r[:, 0:TS], 1e-6)
            nc.scalar.copy(iA, t2i[:, 0:TS])                  # row0 == 0 by basis
            nc.vector.tensor_mul(m1, rA, rA)
            nc.gpsimd.tensor_mul(m2, iA, iA)
            nc.vector.tensor_add(sq, m1, m2)
            inv = work.tile([128, TS], F32, tag="inv")
            nc.scalar.activation(inv, sq, AF.Abs_reciprocal_sqrt, bias=epsb)
            pm = work.tile([128, TS], F32, tag="pm")
            nc.vector.tensor_mul(pm, maga, inv)
            nc.vector.tensor_mul(sa[:, PAD:PAD + TS], rA, pm)
            nc.gpsimd.tensor_mul(sb[:, PAD:PAD + TS], iA, pm)  # row0 -> 0
            # Nyquist sign path (off the critical DVE stretch): mag*sign(Re+eps)
            ge = work.tile([1, TS], F32, tag="ge")
            nc.vector.tensor_scalar(ge, t2n[0:1, 0:TS], -1e-6, 2.0, OP.is_ge, OP.mult)
            nc.vector.scalar_tensor_tensor(sb[0:1, PAD:PAD + TS], ge, 1.0,
                                           magn, OP.subtract, OP.mult)
        if loop is not None:
            loop.__exit__(None, None, None)


_CACHED = None


def _build(rep=1):
    global _CACHED
    if rep == 1 and _CACHED is not None:
        return _CACHED
    nc = bacc.Bacc("TRN2", target_bir_lowering=False, debug=False,
                   num_devices=N_CORES)
    shapes = {
        "a_r": (128, 256), "a_i": (128, 256), "bca": (128, 128),
        "bcb": (128, 128), "bia": (128, 128), "bib": (128, 128),
        "bna": (128, 1), "bnb": (128, 1), "corrt": (32, PAD),
        "maga": (128, TS), "magn": (1, TS),
        "sa0": (128, NW + PAD), "sb0": (128, NW + PAD),
    }
    MMIN = {"a_r", "a_i", "bca", "bcb", "bia", "bib", "bna", "bnb",
            "sa0", "sb0"}
    aps = {name: nc.dram_tensor(name, shape, F32R if name in MMIN else F32,
                                kind="ExternalInput").ap()
           for name, shape in shapes.items()}
    aps["out"] = nc.dram_tensor("out", (32, 32), F32, kind="ExternalOutput").ap()
    with tile.TileContext(nc) as t:
        _emit(t, aps, rep=rep)
    nc.compile()
    if rep == 1:
        _CACHED = nc
    return nc


def _host_inputs(mag_b, ph_b):
    """Per-batch host prep: crop, initial cos/sin spec chunks, padding."""
    a_r, a_i, bc, bi, bn, corrt = _consts()
    mag = np.ascontiguousarray(mag_b[:, :TC]).astype(np.float32)
    ph = np.ascontiguousarray(ph_b[:, :TC]).astype(np.float32)
    sa0 = np.zeros((128, NW + PAD), np.float32)
    sb0 = np.zeros((128, NW + PAD), np.float32)
    sa0[:, PAD:PAD + TC] = mag[0:128] * np.cos(ph[0:128])
    sb0[0, PAD:PAD + TC] = mag[128] * np.cos(ph[128])
    sb0[1:, PAD:PAD + TC] = mag[1:128] * np.sin(ph[1:128])
    return {
        "a_r": a_r, "a_i": a_i,
        "bca": np.ascontiguousarray(bc[0:128]),
        "bcb": np.ascontiguousarray(bc[128:256]),
        "bia": np.ascontiguousarray(bi[0:128]),
        "bib": np.ascontiguousarray(bi[128:256]),
        "bna": np.ascontiguousarray(bn[0:128]),
        "bnb": np.ascontiguousarray(bn[128:256]),
        "corrt": corrt,
        "maga": np.ascontiguousarray(mag[0:128, :TS]),
        "magn": np.ascontiguousarray(mag[128:129, :TS]),
        "sa0": sa0, "sb0": sb0,
    }


def kernel(mag_spec, phase):
    mag_spec = np.asarray(mag_spec, dtype=np.float32)
    phase = np.asarray(phase, dtype=np.float32)
    nc = _build()
    in_maps = [_host_inputs(mag_spec[c % B], phase[c % B]) for c in range(N_CORES)]
    res = bass_utils.run_bass_kernel_spmd(nc, in_maps, core_ids=list(range(N_CORES)))
    out = np.zeros((B, 1000), np.float32)
    for b in range(B):
        blk = res.results[b]["out"]              # (32, 32): [i, m] = wav[32m+i]
        out[b] = blk.T.reshape(-1)[15:1015]
    return out
